# revision 11
# baseline (speedup 1.0000x reference)
"""Trainium2 Bass kernel for the Actor MLP scorer (gnn_message_passing), v2.

Computation (see reference):
    node_e  = node_embeddings[action_nodes]          # [A, 128] gather
    feats   = [node_e | region_embeddings[action_regions] | const_tail]   # [A, 1427]
    h1..h3  = relu MLP (256 wide), logits = h3 @ W4 + b4                  # [A]
    probs   = softmax(logits) over ALL actions

Strategy (8 NeuronCores, data-parallel over actions):
  - Shard A=100000 actions as 12500/core, sorted by node-id bucket
    (< 32768 vs >= 32768) so the node gather can use the int16-indexed
    DMA-gather ucode over two base-offset views of a bf16 table copy.
    transpose=True gather deposits embeddings directly in [dim, action]
    layout (no PE transposes, no PSUM staging).
  - Layer 1 decomposition: feats @ W1 = node_e @ W1[:128]
        + onehot(region) @ (region_embeddings @ W1[128:256])
        + (tail @ W1[256:] + b1)  [host-precomputed constant bias].
    All constant projections (RPS, b1c) are computed on host.
  - Activations stay transposed ([feature, action]); matmuls bf16 with
    fp32 PSUM; relu+bias evictions split across ScalarE/VectorE.
  - No collectives: each core writes its logits; the global softmax
    normalization (exp/sum/divide) happens on host during unsharding.
"""

import sys

for _p in ("/opt/trn_rl_repo",):
    if _p not in sys.path:
        sys.path.insert(0, _p)

import numpy as np
import ml_dtypes
from concourse import bass, bacc, mybir, tile
from concourse import bass_utils
from concourse.masks import make_identity


# ---------------------------------------------------------------- constants
N_CORES = 8
A_FULL = 100000
N_NODES = 50000
N_REGIONS = 8
D = 128
H = 256
G = 147
IN_DIM = 2 * D + N_REGIONS * D + G          # 1427
F32 = mybir.dt.float32
BF16 = mybir.dt.bfloat16
I16 = mybir.dt.int16

A_PC = A_FULL // N_CORES                    # 12500
SPLIT = 32768                               # int16 index range boundary
C0 = 8704                                   # capacity, node id < 32768
C1 = 4608                                   # capacity, node id >= 32768
A_PAD = C0 + C1                             # 13312 = 26*512
ATILE = 512
N_AT = A_PAD // ATILE                       # 26
GCHUNK = 1024                               # idxs per dma_gather call

USE_TGATHER = False                         # dma_gather transpose mode
USE_DMAT = True                             # xbar DMA transpose (vs PE)


def _gather_chunks(total):
    out, off = [], 0
    while off < total:
        n = min(GCHUNK, total - off)
        out.append((off, n))
        off += n
    return out


def build_graph():
    nc = bacc.Bacc("TRN2", target_bir_lowering=False, debug=False,
                   num_devices=N_CORES, num_swdge_queues=4)

    # ---- I/O --------------------------------------------------------------
    node_emb = nc.dram_tensor("node_emb", [N_NODES, D], BF16, kind="ExternalInput")
    wa = nc.dram_tensor("wa", [D, H], BF16, kind="ExternalInput")
    w2b = nc.dram_tensor("w2b", [H, H], BF16, kind="ExternalInput")
    w3b = nc.dram_tensor("w3b", [H, H], BF16, kind="ExternalInput")
    rps_w = nc.dram_tensor("rps_w", [N_REGIONS, H], BF16, kind="ExternalInput")
    w4b = nc.dram_tensor("w4b", [128, 2], BF16, kind="ExternalInput")
    # cols 0:2 b1c | 2:4 b2 | 4:6 b3 | [0,6] b4
    packed = nc.dram_tensor("packed", [128, 8], F32, kind="ExternalInput")
    idx0 = nc.dram_tensor("idx0", [128, C0 // 16], I16, kind="ExternalInput")
    idx1 = nc.dram_tensor("idx1", [128, C1 // 16], I16, kind="ExternalInput")
    onehot = nc.dram_tensor("onehot", [N_REGIONS, A_PAD], BF16, kind="ExternalInput")

    out_logits = nc.dram_tensor("out_logits", [1, A_PAD], F32, kind="ExternalOutput")

    with tile.TileContext(nc) as tc:
        with (
            tc.tile_pool(name="const", bufs=1) as cpool,
            tc.tile_pool(name="hbuf", bufs=2) as hpool,
            tc.tile_pool(name="graw", bufs=6) as gpool,
            tc.tile_pool(name="pnt", bufs=1, space="PSUM") as pnt_pool,
            tc.tile_pool(name="ph", bufs=6 if USE_DMAT else 5,
                         space="PSUM") as ph_pool,
            tc.tile_pool(name="plg", bufs=2, space="PSUM") as plg_pool,
        ):
            # ---- index loads first: gathers depend on them ---------------
            i0 = cpool.tile([128, C0 // 16], I16, tag="i0")
            nc.sync.dma_start(out=i0[:], in_=idx0[:])
            i1 = cpool.tile([128, C1 // 16], I16, tag="i1")
            nc.sync.dma_start(out=i1[:], in_=idx1[:])

            # ---- constant loads (host pre-cast bf16) ----------------------
            w1a = cpool.tile([128, H], BF16, tag="w1a")
            nc.sync.dma_start(out=w1a[:], in_=wa[:])
            rps = cpool.tile([N_REGIONS, H], BF16, tag="rps")
            nc.sync.dma_start(out=rps[:], in_=rps_w[:])
            pk = cpool.tile([128, 8], F32, tag="pk")
            nc.sync.dma_start(out=pk[:], in_=packed[:])
            ohs = cpool.tile([N_REGIONS, A_PAD], BF16, tag="ohs")
            nc.scalar.dma_start(out=ohs[:], in_=onehot[:])
            w2t = [cpool.tile([128, H], BF16, tag=f"w2_{k}", name=f"w2_{k}")
                   for k in range(2)]
            w3t = [cpool.tile([128, H], BF16, tag=f"w3_{k}", name=f"w3_{k}")
                   for k in range(2)]
            for k in range(2):
                nc.scalar.dma_start(out=w2t[k][:], in_=w2b[k * 128:(k + 1) * 128, :])
                nc.scalar.dma_start(out=w3t[k][:], in_=w3b[k * 128:(k + 1) * 128, :])
            w4s = cpool.tile([128, 2], BF16, tag="w4s")
            nc.sync.dma_start(out=w4s[:], in_=w4b[:])

            b1s = pk[:, 0:2]
            b2s = pk[:, 2:4]
            b3s = pk[:, 4:6]
            b4s = pk[0:1, 6:7]

            lrow = cpool.tile([1, A_PAD], F32, tag="lrow")

            # ---- node gather: nts_all[d, slot] = node_emb[id(slot), d] ---
            nts_all = cpool.tile([128, A_PAD], BF16, tag="nts_all")
            gather_plan = (
                [(0, off, n, 0) for off, n in _gather_chunks(C0)]
                + [(C0, off, n, 1) for off, n in _gather_chunks(C1)])

            if not USE_TGATHER and not USE_DMAT:
                ident = cpool.tile([128, 128], BF16, tag="ident")
                make_identity(nc, ident[:])

            # one-time register loads for the gather index counts
            rfull = nc.gpsimd.to_reg(GCHUNK)
            rhalf = nc.gpsimd.to_reg(GCHUNK // 2)

            def emit_gather(gi):
                zone, off, n, grp = gather_plan[gi]
                gsrc = node_emb[0:SPLIT, :] if grp == 0 \
                    else node_emb[SPLIT:N_NODES, :]
                itile = i0 if grp == 0 else i1
                s0 = zone + off
                nreg = rfull if n == GCHUNK else rhalf
                if USE_TGATHER:
                    nc.gpsimd.dma_gather(
                        out_ap=nts_all[:, s0:s0 + n].unsqueeze(1),
                        in_ap=gsrc,
                        idxs_ap=itile[:, off // 16:(off + n) // 16],
                        num_idxs=n, num_idxs_reg=nreg,
                        elem_size=D, transpose=True, single_packet=False,
                        queue_num=1)
                    return n
                graw = gpool.tile([128, n // 128, D], BF16, tag="graw",
                                  name="graw")
                nc.gpsimd.dma_gather(
                    out_ap=graw[:],
                    in_ap=gsrc,
                    idxs_ap=itile[:, off // 16:(off + n) // 16],
                    num_idxs=n, num_idxs_reg=nreg,
                    elem_size=D, transpose=False, single_packet=False,
                    queue_num=1 + (gi % 8) % 3)
                if USE_DMAT:
                    nc.sync.dma_start_transpose(
                        out=nts_all[:, s0:s0 + n].rearrange(
                            "p (c i) -> p c i", i=128),
                        in_=graw[:].rearrange("p c d -> p (c d)"))
                    return n
                nt_ps = pnt_pool.tile([128, GCHUNK], BF16, space="PSUM",
                                      tag="nt_ps", name="nt_ps")
                for c in range(n // 128):
                    nc.tensor.transpose(
                        out=nt_ps[:, c * 128:(c + 1) * 128],
                        in_=graw[:, c, :], identity=ident[:])
                if gi % 2 == 0:
                    nc.scalar.activation(
                        out=nts_all[:, s0:s0 + n], in_=nt_ps[:, 0:n],
                        func=mybir.ActivationFunctionType.Copy)
                else:
                    nc.vector.tensor_copy(out=nts_all[:, s0:s0 + n],
                                          in_=nt_ps[:, 0:n])
                return n

            def evict_relu(engine, dst, src, bias_ap):
                if engine == "act":
                    nc.scalar.activation(
                        out=dst, in_=src,
                        func=mybir.ActivationFunctionType.Relu, bias=bias_ap)
                else:
                    nc.vector.tensor_scalar(
                        out=dst, in0=src, scalar1=bias_ap, scalar2=0.0,
                        op0=mybir.AluOpType.add, op1=mybir.AluOpType.max)

            # ---- main loop: sweeps of 2 action tiles ----------------------
            SWEEP = 2
            t0s = list(range(0, N_AT, SWEEP))
            out_done = 0                       # cols already DMAed out

            def flush_logits(upto):
                nonlocal out_done
                if upto > out_done:
                    nc.sync.dma_start(out=out_logits[0:1, out_done:upto],
                                      in_=lrow[0:1, out_done:upto])
                    out_done = upto

            gi_next = 0
            covered = 0
            # prime the gather pipeline two chunks deep
            while gi_next < len(gather_plan) and covered < 2 * GCHUNK:
                covered += emit_gather(gi_next)
                gi_next += 1
            for si, t0 in enumerate(t0s):
                need = min(t0 + SWEEP, N_AT) * ATILE
                while gi_next < len(gather_plan) and covered < need + GCHUNK:
                    covered += emit_gather(gi_next)
                    gi_next += 1
                tiles = list(range(t0, min(t0 + SWEEP, N_AT)))
                sls = [slice(t * ATILE, (t + 1) * ATILE) for t in tiles]
                nt = len(tiles)

                # layer 1: all node matmuls, then all onehot matmuls, so the
                # PE array config (128x128 vs 8x128) switches once per sweep
                h1 = [[hpool.tile([128, ATILE], BF16, tag=f"h1_{j}_{i}",
                                  name=f"h1_{j}_{i}")
                       for j in range(2)] for i in range(nt)]
                hps1 = [[ph_pool.tile([128, ATILE], F32, space="PSUM",
                                      tag="hps", name="hps")
                         for _ in range(nt)] for _ in range(2)]
                for j in range(2):
                    for i in range(nt):
                        nc.tensor.matmul(out=hps1[j][i][:],
                                         lhsT=w1a[:, j * 128:(j + 1) * 128],
                                         rhs=nts_all[:, sls[i]],
                                         start=True, stop=False)
                for j in range(2):
                    for i in range(nt):
                        nc.tensor.matmul(out=hps1[j][i][:],
                                         lhsT=rps[0:8, j * 128:(j + 1) * 128],
                                         rhs=ohs[0:8, sls[i]],
                                         start=False, stop=True)
                for j in range(2):
                    for i in range(nt):
                        evict_relu("act" if (i + j) % 2 == 0 else "dve",
                                   h1[i][j][:], hps1[j][i][:], b1s[:, j:j + 1])

                # layers 2 and 3
                hin = h1
                for li, (wt, bs) in enumerate(((w2t, b2s), (w3t, b3s))):
                    hout = [[hpool.tile([128, ATILE], BF16,
                                        tag=f"h{li + 2}_{j}_{i}",
                                        name=f"h{li + 2}_{j}_{i}")
                             for j in range(2)] for i in range(nt)]
                    for j in range(2):
                        hps = [ph_pool.tile([128, ATILE], F32, space="PSUM",
                                            tag="hps", name="hps")
                               for _ in range(nt)]
                        for k in range(2):
                            for i in range(nt):
                                nc.tensor.matmul(
                                    out=hps[i][:],
                                    lhsT=wt[k][:, j * 128:(j + 1) * 128],
                                    rhs=hin[i][k][:],
                                    start=(k == 0), stop=(k == 1))
                        for i in range(nt):
                            evict_relu("act" if (i + j + li) % 2 == 0 else "dve",
                                       hout[i][j][:], hps[i][:], bs[:, j:j + 1])
                    hin = hout

                # layer 4: logits — all matmuls batched (one PE config switch)
                lgs = [plg_pool.tile([1, ATILE], F32, space="PSUM", tag="lg",
                                     name="lg") for _ in range(nt)]
                for k in range(2):
                    for i in range(nt):
                        nc.tensor.matmul(out=lgs[i][:], lhsT=w4s[:, k:k + 1],
                                         rhs=hin[i][k][:],
                                         start=(k == 0), stop=(k == 1))
                for i in range(nt):
                    nc.scalar.activation(
                        out=lrow[0:1, sls[i]], in_=lgs[i][:],
                        func=mybir.ActivationFunctionType.Identity,
                        bias=b4s)
                if si in (3, 6, 9):
                    flush_logits((t0 + SWEEP) * ATILE)

            flush_logits(A_PAD)

    nc.compile()
    return nc


_GRAPH_CACHE = {}


def _get_graph():
    if "g" not in _GRAPH_CACHE:
        _GRAPH_CACHE["g"] = build_graph()
    return _GRAPH_CACHE["g"]


def _wrap_idx(ix):
    """int16 index layout for dma_gather: [16, N/16] column-wrapped,
    replicated 8x down the partitions."""
    w = ix.reshape(-1, 16).T
    return np.ascontiguousarray(np.tile(w, (8, 1)))


def make_in_maps(node_embeddings, region_embeddings, global_context,
                 W1, b1, W2, b2, W3, b3, W4, b4,
                 action_nodes, action_regions):
    """Host-side sharding / marshalling. Returns (in_maps, per-core metas)."""
    W1 = np.asarray(W1, np.float32)
    an = np.asarray(action_nodes).astype(np.int64)
    ar = np.asarray(action_regions).astype(np.int64)
    node_bf16 = np.ascontiguousarray(
        np.asarray(node_embeddings, np.float32).astype(ml_dtypes.bfloat16))
    region_embeddings = np.asarray(region_embeddings, np.float32)

    tail = np.concatenate([
        region_embeddings.reshape(-1),
        np.asarray(global_context, np.float32).reshape(-1)])
    b1c = (np.asarray(b1, np.float32)
           + tail @ W1[2 * D:IN_DIM, :]).astype(np.float32)   # [256]
    rps_np = np.ascontiguousarray(
        (region_embeddings @ W1[D:2 * D, :]).astype(ml_dtypes.bfloat16))
    wa_np = np.ascontiguousarray(W1[0:D, :].astype(ml_dtypes.bfloat16))
    w2b_np = np.ascontiguousarray(
        np.asarray(W2, np.float32).astype(ml_dtypes.bfloat16))
    w3b_np = np.ascontiguousarray(
        np.asarray(W3, np.float32).astype(ml_dtypes.bfloat16))
    w4b_np = np.ascontiguousarray(
        np.asarray(W4, np.float32).reshape(2, 128).T.astype(ml_dtypes.bfloat16))

    pk_base = np.zeros((128, 8), np.float32)
    pk_base[:, 0:2] = b1c.reshape(2, 128).T
    pk_base[:, 2:4] = np.asarray(b2, np.float32).reshape(2, 128).T
    pk_base[:, 4:6] = np.asarray(b3, np.float32).reshape(2, 128).T
    pk_base[0, 6] = np.asarray(b4, np.float32).reshape(-1)[0]

    in_maps, metas = [], []
    for c in range(N_CORES):
        s = c * A_PC
        nodes = an[s:s + A_PC]
        regions = ar[s:s + A_PC]
        grp = (nodes >= SPLIT).astype(np.int8)
        order = np.argsort(grp, kind="stable")      # group0 first, stable
        c0 = int((grp == 0).sum())
        c1 = A_PC - c0
        if c0 > C0 or c1 > C1:
            raise RuntimeError(
                f"core {c}: group sizes {c0}/{c1} exceed capacities {C0}/{C1}")
        sn = nodes[order]
        sr = regions[order]

        ix0 = np.zeros(C0, np.int16)
        ix0[:c0] = sn[:c0].astype(np.int16)
        ix1 = np.zeros(C1, np.int16)
        ix1[:c1] = (sn[c0:] - SPLIT).astype(np.int16)

        slots = np.concatenate([np.arange(c0), C0 + np.arange(c1)])
        oh = np.zeros((N_REGIONS, A_PAD), ml_dtypes.bfloat16)
        oh[sr, slots] = 1.0

        in_maps.append({
            "node_emb": node_bf16,
            "wa": wa_np, "w2b": w2b_np, "w3b": w3b_np,
            "rps_w": rps_np, "w4b": w4b_np,
            "packed": pk_base,
            "idx0": _wrap_idx(ix0), "idx1": _wrap_idx(ix1),
            "onehot": oh,
        })
        metas.append((order, slots))
    return in_maps, metas


def _unshard(results, metas):
    logits = np.empty(A_FULL, np.float32)
    for c in range(N_CORES):
        order, slots = metas[c]
        lg = np.asarray(results[c]).reshape(-1)[slots]
        logits[c * A_PC:(c + 1) * A_PC][order] = lg
    le = logits.astype(np.float64)
    e = np.exp(le - le.max())
    probs = (e / e.sum()).astype(np.float32)
    return probs, logits


def kernel(**inputs):
    nc = _get_graph()
    in_maps, metas = make_in_maps(**inputs)
    res = bass_utils.run_bass_kernel_spmd(
        nc, in_maps, core_ids=list(range(N_CORES)))
    return _unshard([res.results[c]["out_logits"] for c in range(N_CORES)],
                    metas)


# revision 17
# speedup vs baseline: 1.8508x; 1.8508x over previous
"""Trainium2 Bass kernel for the Actor MLP scorer (gnn_message_passing), v2.

Computation (see reference):
    node_e  = node_embeddings[action_nodes]          # [A, 128] gather
    feats   = [node_e | region_embeddings[action_regions] | const_tail]   # [A, 1427]
    h1..h3  = relu MLP (256 wide), logits = h3 @ W4 + b4                  # [A]
    probs   = softmax(logits) over ALL actions

Strategy (8 NeuronCores, data-parallel over actions):
  - Shard A=100000 actions as 12500/core, sorted by node-id bucket
    (< 32768 vs >= 32768) so the node gather can use the int16-indexed
    DMA-gather ucode over two base-offset views of a bf16 table copy.
    transpose=True gather deposits embeddings directly in [dim, action]
    layout (no PE transposes, no PSUM staging).
  - Layer 1 decomposition: feats @ W1 = node_e @ W1[:128]
        + onehot(region) @ (region_embeddings @ W1[128:256])
        + (tail @ W1[256:] + b1)  [host-precomputed constant bias].
    All constant projections (RPS, b1c) are computed on host.
  - Activations stay transposed ([feature, action]); matmuls bf16 with
    fp32 PSUM; relu+bias evictions split across ScalarE/VectorE.
  - No collectives: each core writes its logits; the global softmax
    normalization (exp/sum/divide) happens on host during unsharding.
"""

import sys

for _p in ("/opt/trn_rl_repo",):
    if _p not in sys.path:
        sys.path.insert(0, _p)

import numpy as np
import ml_dtypes
from concourse import bass, bacc, mybir, tile
from concourse import bass_utils
from concourse.masks import make_identity


# ---------------------------------------------------------------- constants
N_CORES = 8
A_FULL = 100000
N_NODES = 50000
N_REGIONS = 8
D = 128
H = 256
G = 147
IN_DIM = 2 * D + N_REGIONS * D + G          # 1427
F32 = mybir.dt.float32
BF16 = mybir.dt.bfloat16
I16 = mybir.dt.int16

A_PC = A_FULL // N_CORES                    # 12500
SPLIT = 32768                               # int16 index range boundary
C0 = 8704                                   # capacity, node id < 32768
C1 = 4608                                   # capacity, node id >= 32768
A_PAD = C0 + C1                             # 13312 = 26*512
ATILE = 512
N_AT = A_PAD // ATILE                       # 26
GCHUNK = 1024                               # idxs per dma_gather call

USE_TGATHER = False                         # dma_gather transpose mode
USE_DMAT = False                            # xbar DMA transpose (vs PE)


def _gather_chunks(total):
    out, off = [], 0
    while off < total:
        n = min(GCHUNK, total - off)
        out.append((off, n))
        off += n
    return out


def build_graph():
    nc = bacc.Bacc("TRN2", target_bir_lowering=False, debug=False,
                   num_devices=N_CORES, num_swdge_queues=4)

    # ---- I/O --------------------------------------------------------------
    node_emb = nc.dram_tensor("node_emb", [N_NODES, D], BF16, kind="ExternalInput")
    wa = nc.dram_tensor("wa", [D, H], BF16, kind="ExternalInput")
    w2b = nc.dram_tensor("w2b", [H, H], BF16, kind="ExternalInput")
    w3b = nc.dram_tensor("w3b", [H, H], BF16, kind="ExternalInput")
    rps_w = nc.dram_tensor("rps_w", [N_REGIONS, H], BF16, kind="ExternalInput")
    w4b = nc.dram_tensor("w4b", [128, 2], BF16, kind="ExternalInput")
    identw = nc.dram_tensor("identw", [128, 128], BF16, kind="ExternalInput")
    # cols 0:2 b1c | 2:4 b2 | 4:6 b3 | [0,6] b4
    packed = nc.dram_tensor("packed", [128, 8], F32, kind="ExternalInput")
    idx0 = nc.dram_tensor("idx0", [128, C0 // 16], I16, kind="ExternalInput")
    idx1 = nc.dram_tensor("idx1", [128, C1 // 16], I16, kind="ExternalInput")
    onehot = nc.dram_tensor("onehot", [N_REGIONS, A_PAD], BF16, kind="ExternalInput")

    out_logits = nc.dram_tensor("out_logits", [1, A_PAD], F32, kind="ExternalOutput")

    with tile.TileContext(nc) as tc:
        with (
            tc.tile_pool(name="const", bufs=1) as cpool,
            tc.tile_pool(name="hbuf", bufs=2) as hpool,
            tc.tile_pool(name="graw", bufs=6) as gpool,
            tc.tile_pool(name="pnt", bufs=1, space="PSUM") as pnt_pool,
            tc.tile_pool(name="ph", bufs=6 if USE_DMAT else 5,
                         space="PSUM") as ph_pool,
            tc.tile_pool(name="plg", bufs=2, space="PSUM") as plg_pool,
        ):
            # ---- index loads first: gathers depend on them ---------------
            i0 = cpool.tile([128, C0 // 16], I16, tag="i0")
            nc.sync.dma_start(out=i0[:], in_=idx0[:])
            i1 = cpool.tile([128, C1 // 16], I16, tag="i1")
            nc.sync.dma_start(out=i1[:], in_=idx1[:])

            # ---- constant loads (host pre-cast bf16) ----------------------
            w1a = cpool.tile([128, H], BF16, tag="w1a")
            nc.sync.dma_start(out=w1a[:], in_=wa[:])
            rps = cpool.tile([N_REGIONS, H], BF16, tag="rps")
            nc.sync.dma_start(out=rps[:], in_=rps_w[:])
            pk = cpool.tile([128, 8], F32, tag="pk")
            nc.sync.dma_start(out=pk[:], in_=packed[:])
            ohs = cpool.tile([N_REGIONS, A_PAD], BF16, tag="ohs")
            nc.scalar.dma_start(out=ohs[:], in_=onehot[:])
            w2t = [cpool.tile([128, H], BF16, tag=f"w2_{k}", name=f"w2_{k}")
                   for k in range(2)]
            w3t = [cpool.tile([128, H], BF16, tag=f"w3_{k}", name=f"w3_{k}")
                   for k in range(2)]
            for k in range(2):
                nc.scalar.dma_start(out=w2t[k][:], in_=w2b[k * 128:(k + 1) * 128, :])
                nc.scalar.dma_start(out=w3t[k][:], in_=w3b[k * 128:(k + 1) * 128, :])
            w4s = cpool.tile([128, 2], BF16, tag="w4s")
            nc.sync.dma_start(out=w4s[:], in_=w4b[:])

            b1s = pk[:, 0:2]
            b2s = pk[:, 2:4]
            b3s = pk[:, 4:6]
            b4s = pk[0:1, 6:7]

            lrow = cpool.tile([1, A_PAD], F32, tag="lrow")

            # ---- node gather: nts_all[d, slot] = node_emb[id(slot), d] ---
            nts_all = cpool.tile([128, A_PAD], BF16, tag="nts_all")
            gather_plan = (
                [(0, off, n, 0) for off, n in _gather_chunks(C0)]
                + [(C0, off, n, 1) for off, n in _gather_chunks(C1)])

            if not USE_TGATHER and not USE_DMAT:
                # host-provided identity: keeps the gpsimd queue clear so the
                # Q7 gather-library load starts as early as possible
                ident = cpool.tile([128, 128], BF16, tag="ident")
                nc.sync.dma_start(out=ident[:], in_=identw[:])

            # one-time register loads for the gather index counts
            rfull = nc.gpsimd.to_reg(GCHUNK)
            rhalf = nc.gpsimd.to_reg(GCHUNK // 2)

            def emit_gather(gi):
                zone, off, n, grp = gather_plan[gi]
                gsrc = node_emb[0:SPLIT, :] if grp == 0 \
                    else node_emb[SPLIT:N_NODES, :]
                itile = i0 if grp == 0 else i1
                s0 = zone + off
                nreg = rfull if n == GCHUNK else rhalf
                if USE_TGATHER:
                    nc.gpsimd.dma_gather(
                        out_ap=nts_all[:, s0:s0 + n].unsqueeze(1),
                        in_ap=gsrc,
                        idxs_ap=itile[:, off // 16:(off + n) // 16],
                        num_idxs=n, num_idxs_reg=nreg,
                        elem_size=D, transpose=True, single_packet=False,
                        queue_num=1)
                    return n
                graw = gpool.tile([128, n // 128, D], BF16, tag="graw",
                                  name="graw")
                nc.gpsimd.dma_gather(
                    out_ap=graw[:],
                    in_ap=gsrc,
                    idxs_ap=itile[:, off // 16:(off + n) // 16],
                    num_idxs=n, num_idxs_reg=nreg,
                    elem_size=D, transpose=False, single_packet=False,
                    queue_num=1 + (gi % 8) % 3)
                if USE_DMAT:
                    nc.sync.dma_start_transpose(
                        out=nts_all[:, s0:s0 + n].rearrange(
                            "p (c i) -> p c i", i=128),
                        in_=graw[:].rearrange("p c d -> p (c d)"))
                    return n
                nt_ps = pnt_pool.tile([128, GCHUNK], BF16, space="PSUM",
                                      tag="nt_ps", name="nt_ps")
                for c in range(n // 128):
                    nc.tensor.transpose(
                        out=nt_ps[:, c * 128:(c + 1) * 128],
                        in_=graw[:, c, :], identity=ident[:])
                if gi % 2 == 0:
                    nc.scalar.activation(
                        out=nts_all[:, s0:s0 + n], in_=nt_ps[:, 0:n],
                        func=mybir.ActivationFunctionType.Copy)
                else:
                    nc.vector.tensor_copy(out=nts_all[:, s0:s0 + n],
                                          in_=nt_ps[:, 0:n])
                return n

            def evict_relu(engine, dst, src, bias_ap):
                if engine == "act":
                    nc.scalar.activation(
                        out=dst, in_=src,
                        func=mybir.ActivationFunctionType.Relu, bias=bias_ap)
                else:
                    nc.vector.tensor_scalar(
                        out=dst, in0=src, scalar1=bias_ap, scalar2=0.0,
                        op0=mybir.AluOpType.add, op1=mybir.AluOpType.max)

            # ---- main loop: sweeps of 2 action tiles ----------------------
            SWEEP = 2
            t0s = list(range(0, N_AT, SWEEP))
            out_done = 0                       # cols already DMAed out

            def flush_logits(upto):
                nonlocal out_done
                if upto > out_done:
                    nc.sync.dma_start(out=out_logits[0:1, out_done:upto],
                                      in_=lrow[0:1, out_done:upto])
                    out_done = upto

            gi_next = 0
            covered = 0
            # prime the gather pipeline two chunks deep
            while gi_next < len(gather_plan) and covered < 2 * GCHUNK:
                covered += emit_gather(gi_next)
                gi_next += 1
            for si, t0 in enumerate(t0s):
                need = min(t0 + SWEEP, N_AT) * ATILE
                while gi_next < len(gather_plan) and covered < need + GCHUNK:
                    covered += emit_gather(gi_next)
                    gi_next += 1
                tiles = list(range(t0, min(t0 + SWEEP, N_AT)))
                sls = [slice(t * ATILE, (t + 1) * ATILE) for t in tiles]
                nt = len(tiles)

                # layer 1: all node matmuls, then all onehot matmuls, so the
                # PE array config (128x128 vs 8x128) switches once per sweep
                h1 = [[hpool.tile([128, ATILE], BF16, tag=f"h1_{j}_{i}",
                                  name=f"h1_{j}_{i}")
                       for j in range(2)] for i in range(nt)]
                hps1 = [[ph_pool.tile([128, ATILE], F32, space="PSUM",
                                      tag="hps", name="hps")
                         for _ in range(nt)] for _ in range(2)]
                for j in range(2):
                    for i in range(nt):
                        nc.tensor.matmul(out=hps1[j][i][:],
                                         lhsT=w1a[:, j * 128:(j + 1) * 128],
                                         rhs=nts_all[:, sls[i]],
                                         start=True, stop=False)
                for j in range(2):
                    for i in range(nt):
                        nc.tensor.matmul(out=hps1[j][i][:],
                                         lhsT=rps[0:8, j * 128:(j + 1) * 128],
                                         rhs=ohs[0:8, sls[i]],
                                         start=False, stop=True)
                for j in range(2):
                    for i in range(nt):
                        evict_relu("act" if (i + j) % 2 == 0 else "dve",
                                   h1[i][j][:], hps1[j][i][:], b1s[:, j:j + 1])

                # layers 2 and 3
                hin = h1
                for li, (wt, bs) in enumerate(((w2t, b2s), (w3t, b3s))):
                    hout = [[hpool.tile([128, ATILE], BF16,
                                        tag=f"h{li + 2}_{j}_{i}",
                                        name=f"h{li + 2}_{j}_{i}")
                             for j in range(2)] for i in range(nt)]
                    for j in range(2):
                        hps = [ph_pool.tile([128, ATILE], F32, space="PSUM",
                                            tag="hps", name="hps")
                               for _ in range(nt)]
                        for k in range(2):
                            for i in range(nt):
                                nc.tensor.matmul(
                                    out=hps[i][:],
                                    lhsT=wt[k][:, j * 128:(j + 1) * 128],
                                    rhs=hin[i][k][:],
                                    start=(k == 0), stop=(k == 1))
                        for i in range(nt):
                            evict_relu("act" if (i + j + li) % 2 == 0 else "dve",
                                       hout[i][j][:], hps[i][:], bs[:, j:j + 1])
                    hin = hout

                # layer 4: logits — all matmuls batched (one PE config switch)
                lgs = [plg_pool.tile([1, ATILE], F32, space="PSUM", tag="lg",
                                     name="lg") for _ in range(nt)]
                for k in range(2):
                    for i in range(nt):
                        nc.tensor.matmul(out=lgs[i][:], lhsT=w4s[:, k:k + 1],
                                         rhs=hin[i][k][:],
                                         start=(k == 0), stop=(k == 1))
                for i in range(nt):
                    nc.scalar.activation(
                        out=lrow[0:1, sls[i]], in_=lgs[i][:],
                        func=mybir.ActivationFunctionType.Identity,
                        bias=b4s)
                if si in (3, 6, 9):
                    flush_logits((t0 + SWEEP) * ATILE)

            flush_logits(A_PAD)

    nc.compile()
    return nc


_GRAPH_CACHE = {}


def _get_graph():
    if "g" not in _GRAPH_CACHE:
        _GRAPH_CACHE["g"] = build_graph()
    return _GRAPH_CACHE["g"]


def _wrap_idx(ix):
    """int16 index layout for dma_gather: [16, N/16] column-wrapped,
    replicated 8x down the partitions."""
    w = ix.reshape(-1, 16).T
    return np.ascontiguousarray(np.tile(w, (8, 1)))


def make_in_maps(node_embeddings, region_embeddings, global_context,
                 W1, b1, W2, b2, W3, b3, W4, b4,
                 action_nodes, action_regions):
    """Host-side sharding / marshalling. Returns (in_maps, per-core metas)."""
    W1 = np.asarray(W1, np.float32)
    an = np.asarray(action_nodes).astype(np.int64)
    ar = np.asarray(action_regions).astype(np.int64)
    node_bf16 = np.ascontiguousarray(
        np.asarray(node_embeddings, np.float32).astype(ml_dtypes.bfloat16))
    region_embeddings = np.asarray(region_embeddings, np.float32)

    tail = np.concatenate([
        region_embeddings.reshape(-1),
        np.asarray(global_context, np.float32).reshape(-1)])
    b1c = (np.asarray(b1, np.float32)
           + tail @ W1[2 * D:IN_DIM, :]).astype(np.float32)   # [256]
    rps_np = np.ascontiguousarray(
        (region_embeddings @ W1[D:2 * D, :]).astype(ml_dtypes.bfloat16))
    wa_np = np.ascontiguousarray(W1[0:D, :].astype(ml_dtypes.bfloat16))
    w2b_np = np.ascontiguousarray(
        np.asarray(W2, np.float32).astype(ml_dtypes.bfloat16))
    w3b_np = np.ascontiguousarray(
        np.asarray(W3, np.float32).astype(ml_dtypes.bfloat16))
    w4b_np = np.ascontiguousarray(
        np.asarray(W4, np.float32).reshape(2, 128).T.astype(ml_dtypes.bfloat16))

    pk_base = np.zeros((128, 8), np.float32)
    pk_base[:, 0:2] = b1c.reshape(2, 128).T
    pk_base[:, 2:4] = np.asarray(b2, np.float32).reshape(2, 128).T
    pk_base[:, 4:6] = np.asarray(b3, np.float32).reshape(2, 128).T
    pk_base[0, 6] = np.asarray(b4, np.float32).reshape(-1)[0]

    in_maps, metas = [], []
    for c in range(N_CORES):
        s = c * A_PC
        nodes = an[s:s + A_PC]
        regions = ar[s:s + A_PC]
        grp = (nodes >= SPLIT).astype(np.int8)
        order = np.argsort(grp, kind="stable")      # group0 first, stable
        c0 = int((grp == 0).sum())
        c1 = A_PC - c0
        if c0 > C0 or c1 > C1:
            raise RuntimeError(
                f"core {c}: group sizes {c0}/{c1} exceed capacities {C0}/{C1}")
        sn = nodes[order]
        sr = regions[order]

        ix0 = np.zeros(C0, np.int16)
        ix0[:c0] = sn[:c0].astype(np.int16)
        ix1 = np.zeros(C1, np.int16)
        ix1[:c1] = (sn[c0:] - SPLIT).astype(np.int16)

        slots = np.concatenate([np.arange(c0), C0 + np.arange(c1)])
        oh = np.zeros((N_REGIONS, A_PAD), ml_dtypes.bfloat16)
        oh[sr, slots] = 1.0

        in_maps.append({
            "node_emb": node_bf16,
            "wa": wa_np, "w2b": w2b_np, "w3b": w3b_np,
            "rps_w": rps_np, "w4b": w4b_np,
            "identw": np.eye(128, dtype=ml_dtypes.bfloat16),
            "packed": pk_base,
            "idx0": _wrap_idx(ix0), "idx1": _wrap_idx(ix1),
            "onehot": oh,
        })
        metas.append((order, slots))
    return in_maps, metas


def _unshard(results, metas):
    logits = np.empty(A_FULL, np.float32)
    for c in range(N_CORES):
        order, slots = metas[c]
        lg = np.asarray(results[c]).reshape(-1)[slots]
        logits[c * A_PC:(c + 1) * A_PC][order] = lg
    le = logits.astype(np.float64)
    e = np.exp(le - le.max())
    probs = (e / e.sum()).astype(np.float32)
    return probs, logits


def kernel(**inputs):
    nc = _get_graph()
    in_maps, metas = make_in_maps(**inputs)
    res = bass_utils.run_bass_kernel_spmd(
        nc, in_maps, core_ids=list(range(N_CORES)))
    return _unshard([res.results[c]["out_logits"] for c in range(N_CORES)],
                    metas)
